# revision 8
# baseline (speedup 1.0000x reference)
"""Grouped MLP (8-expert SwiGLU) Trainium2 Bass kernel.

Sharding: expert-parallel, one group per NeuronCore (8 cores).
Token t belongs to group t % 8, so core n gets x[n::8] (4096 tokens),
its expert's gate/up/down weights, and produces out[n::8].

v2 design (from NTFF trace analysis of the fp32r baseline):
- All matmul operands in bf16 (max rel err ~3e-3, gate is 2e-2): halves
  HBM traffic and SBUF footprint, and enables FWL weight loads.
- Stationary-operand reuse x4: each LDWEIGHTS serves 4 matmuls (4 moving
  t-tiles for gate/up, 4 t-tiles for the wd-stationary down projection),
  cutting the per-matmul weight-swap handoff that made the baseline's
  steady-state MM spacing 230ns instead of ~216ns.
- Down projection uses wd as the stationary operand, so hidden stays in
  its natural [h, t] layout as the moving operand and out is produced in
  [O, T] layout (host transposes back).
- Dummy warmup matmuls on a zeroed SBUF tile run during the ~7us
  framework preamble + first-DMA window so the PE's HAM clock gate is
  already at 8/8 when real matmuls start.
- Weights are host-pre-tiled so every DMA descriptor is a contiguous
  2-4KB run; first-block loads are spread across the sync, scalar and
  gpsimd issue queues to shorten the startup critical path.
"""

import sys

if "/opt/trn_rl_repo" not in sys.path:
    sys.path.insert(0, "/opt/trn_rl_repo")

import numpy as np
import ml_dtypes

import concourse.bass as bass  # noqa: F401  (registers bass machinery)
import concourse.tile as tile
from concourse import bacc, mybir
from concourse.bass_utils import run_bass_kernel_spmd

P = 128
T = 4096   # tokens per core (per group)
K = 1024   # d_in
H = 2048   # d_hid
O = 1024   # d_out
N_CORES = 8

F32 = mybir.dt.float32
BF16 = mybir.dt.bfloat16
NPBF = ml_dtypes.bfloat16

# Tiling knobs
TB = 2048           # token block
NTB = T // TB       # 2 token blocks
NT = TB // 512      # 4 moving t-tiles per block
KO = K // P         # 8 k-subtiles
NWC = H // P        # 16 hidden chunks of 128
HO = H // P         # 16 h-subtiles
NOC = O // P        # 8 output column groups
N_WARM = 30         # dummy warmup matmuls at N=256 (~5us of PE busy)

_CACHED_NC = None


def _build_nc():
    from contextlib import ExitStack

    nc = bacc.Bacc(None, target_bir_lowering=False)
    xt = nc.dram_tensor("xt", [K, T], BF16, kind="ExternalInput")
    wg = nc.dram_tensor("wg", [P, NWC, KO, P], BF16, kind="ExternalInput")
    wu = nc.dram_tensor("wu", [P, NWC, KO, P], BF16, kind="ExternalInput")
    wd = nc.dram_tensor("wd", [P, HO, O], BF16, kind="ExternalInput")
    out = nc.dram_tensor("out", [O, T], F32, kind="ExternalOutput")

    silu_fn = mybir.ActivationFunctionType.Silu

    with tile.TileContext(nc) as tc, ExitStack() as ctx:
        const = ctx.enter_context(tc.tile_pool(name="const", bufs=1))
        xpool = ctx.enter_context(tc.tile_pool(name="xpool", bufs=2))
        wpool = ctx.enter_context(tc.tile_pool(name="wpool", bufs=3))
        hpool = ctx.enter_context(tc.tile_pool(name="hpool", bufs=1))
        spool = ctx.enter_context(tc.tile_pool(name="spool", bufs=4))
        opool = ctx.enter_context(tc.tile_pool(name="opool", bufs=2))
        psum = ctx.enter_context(tc.tile_pool(name="psum", bufs=8, space="PSUM"))

        # PE warmup: matmuls with no DMA dependency fill the framework
        # preamble + first-DMA window and flip the HAM clock gate to 8/8
        # before real matmuls arrive.
        dummy_w = const.tile([P, 512], BF16)
        nc.vector.memset(dummy_w[:], 0)
        dps = psum.tile([P, 512], F32, tag="acc")
        for _ in range(N_WARM):
            nc.tensor.matmul(
                dps[:, 0:256], dummy_w[:, 0:P], dummy_w[:, 0:256],
                start=True, stop=True,
            )

        # Down-projection weights: resident for the whole kernel, streamed
        # on the scalar queue interleaved with the up-weight chunks.
        wd_sb = const.tile([P, HO, O], BF16)

        xt_sbs = [None] * NTB
        for tb in range(NTB):
            if tb == 0:
                xt_sbs[0] = xpool.tile([P, KO, TB], BF16, tag="xt", name="xt_sb0")
            xt_sb = xt_sbs[tb]
            hid_sb = hpool.tile([P, HO, TB], BF16, tag="hid")

            for wc in range(NWC):
                wg_sb = wpool.tile([P, KO, P], BF16, tag="wg")
                wu_sb = wpool.tile([P, KO, P], BF16, tag="wu")
                if tb == 0 and wc == 0:
                    # Startup critical path: first weight chunk in three
                    # pieces (first LDW only waits on 32KB) interleaved with
                    # half-ko activation slices on the sync queue; the other
                    # xt half rides gpsimd.  wu0 follows the xt slices.
                    hb = TB // 2
                    nc.sync.dma_start(wg_sb[:, 0:1], wg[:, 0, 0:1])
                    nc.sync.dma_start(xt_sb[:, 0, 0:hb], xt[0:P, 0:hb])
                    nc.sync.dma_start(wg_sb[:, 1:4], wg[:, 0, 1:4])
                    nc.sync.dma_start(xt_sb[:, 0, hb:TB], xt[0:P, hb:TB])
                    nc.sync.dma_start(wg_sb[:, 4:8], wg[:, 0, 4:8])
                    for ko in range(1, 4):
                        r = slice(ko * P, (ko + 1) * P)
                        nc.sync.dma_start(xt_sb[:, ko, 0:hb], xt[r, 0:hb])
                        nc.sync.dma_start(xt_sb[:, ko, hb:TB], xt[r, hb:TB])
                    nc.sync.dma_start(wu_sb[:], wu[:, wc])
                    for ko in range(4, KO):
                        r = slice(ko * P, (ko + 1) * P)
                        nc.gpsimd.dma_start(xt_sb[:, ko, 0:hb], xt[r, 0:hb])
                        nc.gpsimd.dma_start(xt_sb[:, ko, hb:TB], xt[r, hb:TB])
                else:
                    nc.sync.dma_start(wg_sb[:], wg[:, wc])
                    nc.sync.dma_start(wu_sb[:], wu[:, wc])
                if tb == 0:
                    if wc == 1:
                        # Prefetch block 1's activations on the gpsimd queue.
                        xt_sbs[1] = xpool.tile([P, KO, TB], BF16, tag="xt", name="xt_sb1")
                        for ko in range(KO):
                            nc.gpsimd.dma_start(
                                xt_sbs[1][:, ko],
                                xt[ko * P : (ko + 1) * P, TB : 2 * TB],
                            )
                    if wc >= 2:
                        # Down weights ride the gpsimd queue (idle until the
                        # down phase) behind the xt prefetches; two chunks
                        # per wc iteration starting at wc=2.
                        for j in (2 * (wc - 2), 2 * (wc - 2) + 1):
                            if j < HO:
                                nc.gpsimd.dma_start(wd_sb[:, j], wd[:, j])

                # Gate section: one stationary load per (ko), 4 moving
                # t-tiles each, accumulating over ko into 4 PSUM banks.
                gps = [psum.tile([P, 512], F32, tag="acc", name=f"gps{i}") for i in range(NT)]
                if tb == 0 and wc == 0:
                    ko_order = [0, 4, 1, 5, 2, 6, 3, 7]
                else:
                    ko_order = list(range(KO))
                for i, ko in enumerate(ko_order):
                    for th in range(NT):
                        nc.tensor.matmul(
                            gps[th][:],
                            wg_sb[:, ko],
                            xt_sb[:, ko, th * 512 : (th + 1) * 512],
                            start=(i == 0),
                            stop=(i == KO - 1),
                        )
                # Up section.
                ups = [psum.tile([P, 512], F32, tag="acc", name=f"ups{i}") for i in range(NT)]
                for ko in range(KO):
                    for th in range(NT):
                        nc.tensor.matmul(
                            ups[th][:],
                            wu_sb[:, ko],
                            xt_sb[:, ko, th * 512 : (th + 1) * 512],
                            start=(ko == 0),
                            stop=(ko == KO - 1),
                        )
                # SwiGLU epilogue: silu on scalar engine, mul on vector.
                for th in range(NT):
                    tsl = slice(th * 512, (th + 1) * 512)
                    s = spool.tile([P, 512], BF16, tag="silu")
                    nc.scalar.activation(s[:], gps[th][:], silu_fn)
                    nc.vector.tensor_mul(hid_sb[:, wc, tsl], s[:], ups[th][:])

            # Down projection: wd stationary (one load per (oc, ho), 4
            # moving hidden t-tiles), output lands transposed as [O, T].
            for oc in range(NOC):
                osl = slice(oc * P, (oc + 1) * P)
                last = tb == NTB - 1 and oc == NOC - 1
                if last:
                    # Final group: th-outer so each t-tile finishes its
                    # accumulation early and its copy+store overlaps the
                    # next tile's matmuls; stores spread across four issue
                    # queues so the tail is one small parallel transfer.
                    engs = [nc.sync, nc.scalar, nc.gpsimd, nc.sync]
                    for th in range(NT):
                        op_l = psum.tile([P, 512], F32, tag="acc", name="op_l")
                        for ho in range(HO):
                            nc.tensor.matmul(
                                op_l[:],
                                wd_sb[:, ho, osl],
                                hid_sb[:, ho, th * 512 : (th + 1) * 512],
                                start=(ho == 0),
                                stop=(ho == HO - 1),
                            )
                        obl = opool.tile([P, TB // 2], F32, tag="ob", name="obl")
                        nc.vector.tensor_copy(obl[:, 0:512], op_l[:])
                        engs[th].dma_start(
                            out[osl, tb * TB + th * 512 : tb * TB + (th + 1) * 512],
                            obl[:, 0:512],
                        )
                    continue
                ops = [psum.tile([P, 512], F32, tag="acc", name=f"ops{i}") for i in range(NT)]
                for ho in range(HO):
                    for th in range(NT):
                        nc.tensor.matmul(
                            ops[th][:],
                            wd_sb[:, ho, osl],
                            hid_sb[:, ho, th * 512 : (th + 1) * 512],
                            start=(ho == 0),
                            stop=(ho == HO - 1),
                        )
                if True:
                    for half in range(2):
                        ob = opool.tile([P, TB // 2], F32, tag="ob")
                        for th in (2 * half, 2 * half + 1):
                            nc.vector.tensor_copy(
                                ob[:, (th % 2) * 512 : (th % 2 + 1) * 512],
                                ops[th][:],
                            )
                        nc.gpsimd.dma_start(
                            out[
                                osl,
                                tb * TB + half * (TB // 2) : tb * TB
                                + (half + 1) * (TB // 2),
                            ],
                            ob[:],
                        )
                else:
                    # Final group handled separately below.
                    pass

    nc.compile()
    return nc


def _get_nc():
    global _CACHED_NC
    if _CACHED_NC is None:
        _CACHED_NC = _build_nc()
    return _CACHED_NC


def _make_in_maps(x, gate_weight, up_weight, down_weight, n):
    in_maps = []
    for g in range(n):
        xt = np.ascontiguousarray(x[g::n].T.astype(NPBF))
        # [K, H] -> [p, wc, ko, 128] so each weight-chunk DMA descriptor is
        # a contiguous 2KB run per partition.
        wgt = np.ascontiguousarray(
            gate_weight[g].astype(NPBF).reshape(KO, P, NWC, P).transpose(1, 2, 0, 3)
        )
        wut = np.ascontiguousarray(
            up_weight[g].astype(NPBF).reshape(KO, P, NWC, P).transpose(1, 2, 0, 3)
        )
        # [H, O] -> [p, ho, O]
        wdt = np.ascontiguousarray(
            down_weight[g].astype(NPBF).reshape(HO, P, O).transpose(1, 0, 2)
        )
        in_maps.append({"xt": xt, "wg": wgt, "wu": wut, "wd": wdt})
    return in_maps


def _run_spmd(in_maps, **kwargs):
    nc = _get_nc()
    return run_bass_kernel_spmd(nc, in_maps, core_ids=list(range(N_CORES)), **kwargs)


def kernel(x, gate_weight, up_weight, down_weight, num_groups=8):
    n = int(num_groups)
    x = np.asarray(x, dtype=np.float32)
    gate_weight = np.asarray(gate_weight, dtype=np.float32)
    up_weight = np.asarray(up_weight, dtype=np.float32)
    down_weight = np.asarray(down_weight, dtype=np.float32)

    assert n == N_CORES, f"expected {N_CORES} groups, got {n}"
    assert x.shape == (T * N_CORES, K), x.shape
    assert gate_weight.shape == (n, K, H), gate_weight.shape
    assert up_weight.shape == (n, K, H), up_weight.shape
    assert down_weight.shape == (n, H, O), down_weight.shape

    in_maps = _make_in_maps(x, gate_weight, up_weight, down_weight, n)
    res = _run_spmd(in_maps)

    out = np.empty((x.shape[0], O), dtype=np.float32)
    for g in range(n):
        out[g::n] = res.results[g]["out"].T
    return out


# revision 9
# speedup vs baseline: 1.0127x; 1.0127x over previous
"""Grouped MLP (8-expert SwiGLU) Trainium2 Bass kernel.

Sharding: expert-parallel, one group per NeuronCore (8 cores).
Token t belongs to group t % 8, so core n gets x[n::8] (4096 tokens),
its expert's gate/up/down weights, and produces out[n::8].

Design (from NTFF trace analysis):
- All matmul operands in bf16 (max rel err ~4e-3 vs the 2e-2 gate):
  halves HBM traffic and SBUF footprint, and enables FWL weight loads
  (LDWEIGHTS 97ns, fully hidden under the 216ns matmul beat).
- Down projection uses wd as the stationary operand, so hidden stays in
  its natural [h, t] layout as the moving operand and out is produced in
  [O, T] layout (host transposes back).
- Dummy warmup matmuls (N=256, no DMA deps) run during the ~7us
  framework preamble + first-DMA-dead window so the PE's HAM clock gate
  is at 8/8 when real matmuls start, and real matmuls are never the ones
  paying the cold-clock penalty.
- Token blocks of 1024 keep the first block's DMA footprint small
  (wg0 + 8 x 256KB xt slices) so the PE reaches steady state ~15us in;
  startup loads are spread across the sync and gpsimd issue queues,
  weight streams ride sync, bulk prefetches and output stores ride
  gpsimd, and the scalar queue carries only silu activations (a DMA
  issued behind activations deadlocks the ramp into data starvation).
- One shared 8-bank PSUM rotation (tag "acc") across gate/up/down.
"""

import sys

if "/opt/trn_rl_repo" not in sys.path:
    sys.path.insert(0, "/opt/trn_rl_repo")

import numpy as np
import ml_dtypes

import concourse.bass as bass  # noqa: F401  (registers bass machinery)
import concourse.tile as tile
from concourse import bacc, mybir
from concourse.bass_utils import run_bass_kernel_spmd

P = 128
T = 4096   # tokens per core (per group)
K = 1024   # d_in
H = 2048   # d_hid
O = 1024   # d_out
N_CORES = 8

F32 = mybir.dt.float32
BF16 = mybir.dt.bfloat16
NPBF = ml_dtypes.bfloat16

# Tiling knobs
TB = 1024           # token block
NTB = T // TB       # 4 token blocks
NT = TB // 512      # 2 moving t-tiles per block
KO = K // P         # 8 k-subtiles
NWC = H // P        # 16 hidden chunks of 128
HO = H // P         # 16 h-subtiles
NOC = O // P        # 8 output column groups
N_WARM = 24         # dummy warmup matmuls at N=256

_CACHED_NC = None


def _build_nc():
    from contextlib import ExitStack

    nc = bacc.Bacc(None, target_bir_lowering=False)
    xt = nc.dram_tensor("xt", [K, T], BF16, kind="ExternalInput")
    wg = nc.dram_tensor("wg", [P, NWC, KO, P], BF16, kind="ExternalInput")
    wu = nc.dram_tensor("wu", [P, NWC, KO, P], BF16, kind="ExternalInput")
    wd = nc.dram_tensor("wd", [P, HO, O], BF16, kind="ExternalInput")
    out = nc.dram_tensor("out", [O, T], F32, kind="ExternalOutput")

    silu_fn = mybir.ActivationFunctionType.Silu

    with tile.TileContext(nc) as tc, ExitStack() as ctx:
        const = ctx.enter_context(tc.tile_pool(name="const", bufs=1))
        xpool = ctx.enter_context(tc.tile_pool(name="xpool", bufs=2))
        wpool = ctx.enter_context(tc.tile_pool(name="wpool", bufs=3))
        hpool = ctx.enter_context(tc.tile_pool(name="hpool", bufs=2))
        spool = ctx.enter_context(tc.tile_pool(name="spool", bufs=4))
        opool = ctx.enter_context(tc.tile_pool(name="opool", bufs=3))
        psum = ctx.enter_context(tc.tile_pool(name="psum", bufs=8, space="PSUM"))

        # PE warmup (HAM clock gate) — no DMA dependencies.
        dummy_w = const.tile([P, 512], BF16)
        nc.vector.memset(dummy_w[:], 0)
        dps = psum.tile([P, 512], F32, tag="acc")
        for _ in range(N_WARM):
            nc.tensor.matmul(
                dps[:, 0:256], dummy_w[:, 0:P], dummy_w[:, 0:256],
                start=True, stop=True,
            )

        # Down-projection weights: resident for the whole kernel.
        wd_sb = const.tile([P, HO, O], BF16)

        xt_sbs = [None] * NTB
        xt_sbs[0] = xpool.tile([P, KO, TB], BF16, tag="xt", name="xt_sb0")

        for tb in range(NTB):
            xt_sb = xt_sbs[tb]
            hid_sb = hpool.tile([P, HO, TB], BF16, tag="hid")

            for wc in range(NWC):
                wg_sb = wpool.tile([P, KO, P], BF16, tag="wg")
                wu_sb = wpool.tile([P, KO, P], BF16, tag="wu")
                if tb == 0 and wc == 0:
                    # Startup critical path: first weight piece is 32KB so
                    # the first LDWEIGHTS unblocks early; xt slices split
                    # across the sync and gpsimd queues.
                    nc.sync.dma_start(wg_sb[:, 0:1], wg[:, 0, 0:1])
                    nc.sync.dma_start(xt_sb[:, 0], xt[0:P, 0:TB])
                    nc.sync.dma_start(wg_sb[:, 1:8], wg[:, 0, 1:8])
                    for ko in range(1, 4):
                        nc.sync.dma_start(
                            xt_sb[:, ko], xt[ko * P : (ko + 1) * P, 0:TB]
                        )
                    nc.sync.dma_start(wu_sb[:], wu[:, wc])
                    for ko in range(4, KO):
                        nc.gpsimd.dma_start(
                            xt_sb[:, ko], xt[ko * P : (ko + 1) * P, 0:TB]
                        )
                else:
                    nc.sync.dma_start(wg_sb[:], wg[:, wc])
                    nc.sync.dma_start(wu_sb[:], wu[:, wc])
                if tb == 0 and wc >= 2:
                    # Down weights ride the gpsimd queue, two chunks per wc.
                    for j in (2 * (wc - 2), 2 * (wc - 2) + 1):
                        if j < HO:
                            nc.gpsimd.dma_start(wd_sb[:, j], wd[:, j])
                if tb < NTB - 1 and wc == 4:
                    # Prefetch next block's activations on the gpsimd queue.
                    xt_sbs[tb + 1] = xpool.tile(
                        [P, KO, TB], BF16, tag="xt", name=f"xt_sb{tb + 1}"
                    )
                    for ko in range(KO):
                        nc.gpsimd.dma_start(
                            xt_sbs[tb + 1][:, ko],
                            xt[ko * P : (ko + 1) * P, (tb + 1) * TB : (tb + 2) * TB],
                        )

                # Gate section.
                gps = [
                    psum.tile([P, 512], F32, tag="acc", name=f"gps{i}")
                    for i in range(NT)
                ]
                if tb == 0 and wc == 0:
                    ko_order = [0, 4, 1, 5, 2, 6, 3, 7]
                else:
                    ko_order = list(range(KO))
                for i, ko in enumerate(ko_order):
                    for th in range(NT):
                        nc.tensor.matmul(
                            gps[th][:],
                            wg_sb[:, ko],
                            xt_sb[:, ko, th * 512 : (th + 1) * 512],
                            start=(i == 0),
                            stop=(i == KO - 1),
                        )
                # Up section.
                ups = [
                    psum.tile([P, 512], F32, tag="acc", name=f"ups{i}")
                    for i in range(NT)
                ]
                for i, ko in enumerate(ko_order):
                    for th in range(NT):
                        nc.tensor.matmul(
                            ups[th][:],
                            wu_sb[:, ko],
                            xt_sb[:, ko, th * 512 : (th + 1) * 512],
                            start=(i == 0),
                            stop=(i == KO - 1),
                        )
                # SwiGLU epilogue: silu on scalar engine, mul on vector.
                for th in range(NT):
                    tsl = slice(th * 512, (th + 1) * 512)
                    s = spool.tile([P, 512], BF16, tag="silu")
                    nc.scalar.activation(s[:], gps[th][:], silu_fn)
                    nc.vector.tensor_mul(hid_sb[:, wc, tsl], s[:], ups[th][:])

            # Down projection: wd stationary, hidden moving in [h, t]
            # layout; out lands as [O, T].
            for oc in range(NOC):
                osl = slice(oc * P, (oc + 1) * P)
                last = tb == NTB - 1 and oc == NOC - 1
                if last:
                    # Final group: t-tile-outer so each tile's copy+store
                    # overlaps the next tile's matmuls; stores go to two
                    # idle issue queues so the tail is one small transfer.
                    engs = [nc.sync, nc.scalar]
                    for th in range(NT):
                        op_l = psum.tile([P, 512], F32, tag="acc", name="op_l")
                        for ho in range(HO):
                            nc.tensor.matmul(
                                op_l[:],
                                wd_sb[:, ho, osl],
                                hid_sb[:, ho, th * 512 : (th + 1) * 512],
                                start=(ho == 0),
                                stop=(ho == HO - 1),
                            )
                        obl = opool.tile([P, 512], F32, tag="ob", name="obl")
                        nc.vector.tensor_copy(obl[:], op_l[:])
                        engs[th].dma_start(
                            out[osl, tb * TB + th * 512 : tb * TB + (th + 1) * 512],
                            obl[:],
                        )
                    continue
                ops = [
                    psum.tile([P, 512], F32, tag="acc", name=f"ops{i}")
                    for i in range(NT)
                ]
                for ho in range(HO):
                    for th in range(NT):
                        nc.tensor.matmul(
                            ops[th][:],
                            wd_sb[:, ho, osl],
                            hid_sb[:, ho, th * 512 : (th + 1) * 512],
                            start=(ho == 0),
                            stop=(ho == HO - 1),
                        )
                ob = opool.tile([P, TB], F32, tag="obb", name="ob")
                for th in range(NT):
                    nc.vector.tensor_copy(ob[:, th * 512 : (th + 1) * 512], ops[th][:])
                nc.gpsimd.dma_start(out[osl, tb * TB : (tb + 1) * TB], ob[:])

    nc.compile()
    return nc


def _get_nc():
    global _CACHED_NC
    if _CACHED_NC is None:
        _CACHED_NC = _build_nc()
    return _CACHED_NC


def _make_in_maps(x, gate_weight, up_weight, down_weight, n):
    in_maps = []
    for g in range(n):
        xtg = np.ascontiguousarray(x[g::n].T.astype(NPBF))
        # [K, H] -> [p, wc, ko, 128] so each weight-chunk DMA descriptor is
        # a contiguous 2KB run per partition.
        wgt = np.ascontiguousarray(
            gate_weight[g].astype(NPBF).reshape(KO, P, NWC, P).transpose(1, 2, 0, 3)
        )
        wut = np.ascontiguousarray(
            up_weight[g].astype(NPBF).reshape(KO, P, NWC, P).transpose(1, 2, 0, 3)
        )
        # [H, O] -> [p, ho, O]
        wdt = np.ascontiguousarray(
            down_weight[g].astype(NPBF).reshape(HO, P, O).transpose(1, 0, 2)
        )
        in_maps.append({"xt": xtg, "wg": wgt, "wu": wut, "wd": wdt})
    return in_maps


def _run_spmd(in_maps, **kwargs):
    nc = _get_nc()
    return run_bass_kernel_spmd(nc, in_maps, core_ids=list(range(N_CORES)), **kwargs)


def kernel(x, gate_weight, up_weight, down_weight, num_groups=8):
    n = int(num_groups)
    x = np.asarray(x, dtype=np.float32)
    gate_weight = np.asarray(gate_weight, dtype=np.float32)
    up_weight = np.asarray(up_weight, dtype=np.float32)
    down_weight = np.asarray(down_weight, dtype=np.float32)

    assert n == N_CORES, f"expected {N_CORES} groups, got {n}"
    assert x.shape == (T * N_CORES, K), x.shape
    assert gate_weight.shape == (n, K, H), gate_weight.shape
    assert up_weight.shape == (n, K, H), up_weight.shape
    assert down_weight.shape == (n, H, O), down_weight.shape

    in_maps = _make_in_maps(x, gate_weight, up_weight, down_weight, n)
    res = _run_spmd(in_maps)

    out = np.empty((x.shape[0], O), dtype=np.float32)
    for g in range(n):
        out[g::n] = res.results[g]["out"].T
    return out


# revision 10
# speedup vs baseline: 1.0169x; 1.0041x over previous
"""Grouped MLP (8-expert SwiGLU) Trainium2 Bass kernel.

Sharding: expert-parallel, one group per NeuronCore (8 cores).
Token t belongs to group t % 8, so core n gets x[n::8] (4096 tokens),
its expert's gate/up/down weights, and produces out[n::8].

Design (from NTFF trace analysis):
- All matmul operands in bf16 (max rel err ~4e-3 vs the 2e-2 gate):
  halves HBM traffic and SBUF footprint, and enables FWL weight loads
  (LDWEIGHTS 97ns, fully hidden under the 216ns matmul beat).
- Down projection uses wd as the stationary operand, so hidden stays in
  its natural [h, t] layout as the moving operand and out is produced in
  [O, T] layout (host transposes back).
- Dummy warmup matmuls (N=256, no DMA deps) run during the ~7us
  framework preamble + first-DMA-dead window so the PE's HAM clock gate
  is at 8/8 when real matmuls start, and real matmuls are never the ones
  paying the cold-clock penalty.
- Token blocks of 1024 keep the first block's DMA footprint small
  (wg0 + 8 x 256KB xt slices) so the PE reaches steady state ~15us in;
  startup loads are spread across the sync and gpsimd issue queues,
  weight streams ride sync, bulk prefetches and output stores ride
  gpsimd, and the scalar queue carries only silu activations (a DMA
  issued behind activations deadlocks the ramp into data starvation).
- One shared 8-bank PSUM rotation (tag "acc") across gate/up/down.
"""

import sys

if "/opt/trn_rl_repo" not in sys.path:
    sys.path.insert(0, "/opt/trn_rl_repo")

import numpy as np
import ml_dtypes

import concourse.bass as bass  # noqa: F401  (registers bass machinery)
import concourse.tile as tile
from concourse import bacc, mybir
from concourse.bass_utils import run_bass_kernel_spmd

P = 128
T = 4096   # tokens per core (per group)
K = 1024   # d_in
H = 2048   # d_hid
O = 1024   # d_out
N_CORES = 8

F32 = mybir.dt.float32
BF16 = mybir.dt.bfloat16
NPBF = ml_dtypes.bfloat16

# Tiling knobs
TB = 1024           # token block
NTB = T // TB       # 4 token blocks
NT = TB // 512      # 2 moving t-tiles per block
KO = K // P         # 8 k-subtiles
NWC = H // P        # 16 hidden chunks of 128
HO = H // P         # 16 h-subtiles
NOC = O // P        # 8 output column groups
N_WARM = 24         # dummy warmup matmuls at N=256

_CACHED_NC = None


def _build_nc():
    from contextlib import ExitStack

    nc = bacc.Bacc(None, target_bir_lowering=False)
    xt = nc.dram_tensor("xt", [K, T], BF16, kind="ExternalInput")
    wg = nc.dram_tensor("wg", [P, NWC, KO, P], BF16, kind="ExternalInput")
    wu = nc.dram_tensor("wu", [P, NWC, KO, P], BF16, kind="ExternalInput")
    wd = nc.dram_tensor("wd", [P, HO, O], BF16, kind="ExternalInput")
    out = nc.dram_tensor("out", [O, T], F32, kind="ExternalOutput")

    silu_fn = mybir.ActivationFunctionType.Silu

    with tile.TileContext(nc) as tc, ExitStack() as ctx:
        const = ctx.enter_context(tc.tile_pool(name="const", bufs=1))
        xpool = ctx.enter_context(tc.tile_pool(name="xpool", bufs=2))
        wpool = ctx.enter_context(tc.tile_pool(name="wpool", bufs=3))
        hpool = ctx.enter_context(tc.tile_pool(name="hpool", bufs=2))
        spool = ctx.enter_context(tc.tile_pool(name="spool", bufs=4))
        opool = ctx.enter_context(tc.tile_pool(name="opool", bufs=3))
        psum = ctx.enter_context(tc.tile_pool(name="psum", bufs=8, space="PSUM"))

        # PE warmup (HAM clock gate) — no DMA dependencies.
        dummy_w = const.tile([P, 512], BF16)
        nc.vector.memset(dummy_w[:], 0)
        dps = psum.tile([P, 512], F32, tag="acc")
        for _ in range(N_WARM):
            nc.tensor.matmul(
                dps[:, 0:256], dummy_w[:, 0:P], dummy_w[:, 0:256],
                start=True, stop=True,
            )

        # Down-projection weights: resident for the whole kernel.
        wd_sb = const.tile([P, HO, O], BF16)

        xt_sbs = [None] * NTB
        xt_sbs[0] = xpool.tile([P, KO, TB], BF16, tag="xt", name="xt_sb0")

        for tb in range(NTB):
            xt_sb = xt_sbs[tb]
            hid_sb = hpool.tile([P, HO, TB], BF16, tag="hid")

            for wc in range(NWC):
                wg_sb = wpool.tile([P, KO, P], BF16, tag="wg")
                wu_sb = wpool.tile([P, KO, P], BF16, tag="wu")
                if tb == 0 and wc == 0:
                    # Startup critical path: first weight piece is 32KB so
                    # the first LDWEIGHTS unblocks early; xt slices spread
                    # across all three issue queues (sync / scalar /
                    # gpsimd) so their completion semaphores fire early.
                    # The scalar-queue loads are safe only because they
                    # precede every silu activation in program order.
                    nc.sync.dma_start(wg_sb[:, 0:1], wg[:, 0, 0:1])
                    nc.sync.dma_start(xt_sb[:, 0], xt[0:P, 0:TB])
                    nc.sync.dma_start(wg_sb[:, 1:8], wg[:, 0, 1:8])
                    nc.sync.dma_start(xt_sb[:, 1], xt[P : 2 * P, 0:TB])
                    nc.sync.dma_start(xt_sb[:, 2], xt[2 * P : 3 * P, 0:TB])
                    for ko in (3, 5):
                        nc.scalar.dma_start(
                            xt_sb[:, ko], xt[ko * P : (ko + 1) * P, 0:TB]
                        )
                    nc.scalar.dma_start(wu_sb[:], wu[:, wc])
                    for ko in (4, 6, 7):
                        nc.gpsimd.dma_start(
                            xt_sb[:, ko], xt[ko * P : (ko + 1) * P, 0:TB]
                        )
                else:
                    nc.sync.dma_start(wg_sb[:], wg[:, wc])
                    nc.sync.dma_start(wu_sb[:], wu[:, wc])
                if tb == 0 and wc >= 2:
                    # Down weights ride the gpsimd queue, two chunks per wc.
                    for j in (2 * (wc - 2), 2 * (wc - 2) + 1):
                        if j < HO:
                            nc.gpsimd.dma_start(wd_sb[:, j], wd[:, j])
                if tb < NTB - 1 and wc == 4:
                    # Prefetch next block's activations on the gpsimd queue.
                    xt_sbs[tb + 1] = xpool.tile(
                        [P, KO, TB], BF16, tag="xt", name=f"xt_sb{tb + 1}"
                    )
                    for ko in range(KO):
                        nc.gpsimd.dma_start(
                            xt_sbs[tb + 1][:, ko],
                            xt[ko * P : (ko + 1) * P, (tb + 1) * TB : (tb + 2) * TB],
                        )

                # Gate section.
                gps = [
                    psum.tile([P, 512], F32, tag="acc", name=f"gps{i}")
                    for i in range(NT)
                ]
                if tb == 0 and wc == 0:
                    ko_order = [0, 3, 4, 1, 5, 6, 2, 7]
                else:
                    ko_order = list(range(KO))
                for i, ko in enumerate(ko_order):
                    for th in range(NT):
                        nc.tensor.matmul(
                            gps[th][:],
                            wg_sb[:, ko],
                            xt_sb[:, ko, th * 512 : (th + 1) * 512],
                            start=(i == 0),
                            stop=(i == KO - 1),
                        )
                # Up section.
                ups = [
                    psum.tile([P, 512], F32, tag="acc", name=f"ups{i}")
                    for i in range(NT)
                ]
                for i, ko in enumerate(ko_order):
                    for th in range(NT):
                        nc.tensor.matmul(
                            ups[th][:],
                            wu_sb[:, ko],
                            xt_sb[:, ko, th * 512 : (th + 1) * 512],
                            start=(i == 0),
                            stop=(i == KO - 1),
                        )
                # SwiGLU epilogue: silu on scalar engine, mul on vector.
                for th in range(NT):
                    tsl = slice(th * 512, (th + 1) * 512)
                    s = spool.tile([P, 512], BF16, tag="silu")
                    nc.scalar.activation(s[:], gps[th][:], silu_fn)
                    nc.vector.tensor_mul(hid_sb[:, wc, tsl], s[:], ups[th][:])

            # Down projection: wd stationary, hidden moving in [h, t]
            # layout; out lands as [O, T].
            for oc in range(NOC):
                osl = slice(oc * P, (oc + 1) * P)
                last = tb == NTB - 1 and oc == NOC - 1
                if last:
                    # Final group: t-tile-outer so each tile's copy+store
                    # overlaps the next tile's matmuls; stores go to two
                    # idle issue queues so the tail is one small transfer.
                    engs = [nc.sync, nc.scalar]
                    for th in range(NT):
                        op_l = psum.tile([P, 512], F32, tag="acc", name="op_l")
                        for ho in range(HO):
                            nc.tensor.matmul(
                                op_l[:],
                                wd_sb[:, ho, osl],
                                hid_sb[:, ho, th * 512 : (th + 1) * 512],
                                start=(ho == 0),
                                stop=(ho == HO - 1),
                            )
                        obl = opool.tile([P, 512], F32, tag="ob", name="obl")
                        nc.vector.tensor_copy(obl[:], op_l[:])
                        engs[th].dma_start(
                            out[osl, tb * TB + th * 512 : tb * TB + (th + 1) * 512],
                            obl[:],
                        )
                    continue
                ops = [
                    psum.tile([P, 512], F32, tag="acc", name=f"ops{i}")
                    for i in range(NT)
                ]
                for ho in range(HO):
                    for th in range(NT):
                        nc.tensor.matmul(
                            ops[th][:],
                            wd_sb[:, ho, osl],
                            hid_sb[:, ho, th * 512 : (th + 1) * 512],
                            start=(ho == 0),
                            stop=(ho == HO - 1),
                        )
                ob = opool.tile([P, TB], F32, tag="obb", name="ob")
                for th in range(NT):
                    nc.vector.tensor_copy(ob[:, th * 512 : (th + 1) * 512], ops[th][:])
                nc.gpsimd.dma_start(out[osl, tb * TB : (tb + 1) * TB], ob[:])

    nc.compile()
    return nc


def _get_nc():
    global _CACHED_NC
    if _CACHED_NC is None:
        _CACHED_NC = _build_nc()
    return _CACHED_NC


def _make_in_maps(x, gate_weight, up_weight, down_weight, n):
    in_maps = []
    for g in range(n):
        xtg = np.ascontiguousarray(x[g::n].T.astype(NPBF))
        # [K, H] -> [p, wc, ko, 128] so each weight-chunk DMA descriptor is
        # a contiguous 2KB run per partition.
        wgt = np.ascontiguousarray(
            gate_weight[g].astype(NPBF).reshape(KO, P, NWC, P).transpose(1, 2, 0, 3)
        )
        wut = np.ascontiguousarray(
            up_weight[g].astype(NPBF).reshape(KO, P, NWC, P).transpose(1, 2, 0, 3)
        )
        # [H, O] -> [p, ho, O]
        wdt = np.ascontiguousarray(
            down_weight[g].astype(NPBF).reshape(HO, P, O).transpose(1, 0, 2)
        )
        in_maps.append({"xt": xtg, "wg": wgt, "wu": wut, "wd": wdt})
    return in_maps


def _run_spmd(in_maps, **kwargs):
    nc = _get_nc()
    return run_bass_kernel_spmd(nc, in_maps, core_ids=list(range(N_CORES)), **kwargs)


def kernel(x, gate_weight, up_weight, down_weight, num_groups=8):
    n = int(num_groups)
    x = np.asarray(x, dtype=np.float32)
    gate_weight = np.asarray(gate_weight, dtype=np.float32)
    up_weight = np.asarray(up_weight, dtype=np.float32)
    down_weight = np.asarray(down_weight, dtype=np.float32)

    assert n == N_CORES, f"expected {N_CORES} groups, got {n}"
    assert x.shape == (T * N_CORES, K), x.shape
    assert gate_weight.shape == (n, K, H), gate_weight.shape
    assert up_weight.shape == (n, K, H), up_weight.shape
    assert down_weight.shape == (n, H, O), down_weight.shape

    in_maps = _make_in_maps(x, gate_weight, up_weight, down_weight, n)
    res = _run_spmd(in_maps)

    out = np.empty((x.shape[0], O), dtype=np.float32)
    for g in range(n):
        out[g::n] = res.results[g]["out"].T
    return out
